# revision 1
# baseline (speedup 1.0000x reference)
"""MixedQLinear Trainium2 kernel — token-parallel version.

Computation (per reference):
  x2 = x[0]                                  (M=4096, IN_F=4096) fp16
  int_x = x2[:, int_indices]                 (M, 3840)
  fp_x  = x2[:, fp_indices]                  (M, 256)
  per-token asym quant of int_x to int4:  scale=(mx-mn)/15, zero=mn
  q = round((int_x-zero)/scale) - 8          in [-8,7]
  out = scale*w_scale*(q @ w_int.T) + (zero+8*scale)*reduced_w + fp_x@fp_w.T + bias

Strategy: shard TOKENS across the 8 cores (512 each); every core holds
the full out_features dimension.  No collective is needed: each core
computes its own tokens' quant params, quantizes 4 token tiles, and
runs the full matmul against all 4096 out features (8 column groups of
512, weight groups streamed from HBM).

Device-side per core:
  Phase A (4 token tiles):
    - min/max stats (DVE), quant params [scale, rs, bq, alpha],
    - quantize via ScalarE activation (per-partition scale/bias) + fp32
      magic-constant RNE rounding on DVE -> qa f16 (alpha appended as an
      extra column),
    - DMA xbar transpose (128 x 3968 f16) -> k-major qtb,
    - cast to fp8 (alternating DVE/ACT) -> q8 tiles (kept all phase B),
    - alpha row copied from the transposed tile into a tiny (2,512)
      stationary (row0 = alpha, row1 = ones).
  Phase B (8 out-feature groups x 4 token tiles):
    - int path: 15 fp8 DoubleRow matmuls (K=3840), moving = raw int4
      weights (exact in fp8e4),
    - fp path: 1 fp8 DoubleRow matmul (256 fp features) + 1 K=2 f16
      matmul ([alpha; ones] x [redw; bias]) into a second psum,
    - combine: out = (p0*wscale)*scale_t + p1  (two DVE ops),
    - store (128, 512) chunk.

Host side does only layout work: column gather, int4 unpack, fp8 casts,
weight reshapes, token slicing, and concat of per-core outputs.
"""

import os
import sys

import numpy as np

for _p in ("/opt/trn_rl_repo",):
    if _p not in sys.path and os.path.isdir(_p):
        sys.path.insert(0, _p)

TOKENS = 4096
IN_F = 4096
OUT_F = 4096
FP_F = 256
INT_F = IN_F - FP_F          # 3840
NCORES = 8
TPC = TOKENS // NCORES       # 512 tokens per core
NT = TPC // 128              # 4 token tiles per core
KP = INT_F + 128             # 3968 = 31*128 (alpha col + pad)
KE = KP // 128               # 31 k-tiles (30 int + 1 alpha)
NG = 8                       # out-feature groups per core
OG = OUT_F // NG             # 512 out features per group
C_MAGIC = 12582912.0         # 1.5*2^23: fp32 add/sub forces RNE-to-integer

_PROGRAM = None
LAST_RESULTS = None


def _ensure_ntff_hook():
    """Install the axon NTFF profiling hook if the image's antenv lacks it.

    Best-effort: profiling only; compile/run work without it.
    """
    import contextlib
    import ctypes
    import types

    try:
        try:
            import antenv.axon_hooks as hooks_mod
        except ImportError:
            import antenv

            hooks_mod = types.ModuleType("antenv.axon_hooks")
            _holder = {}
            hooks_mod.set_axon_ntff_profile_hook = (
                lambda hook: _holder.__setitem__("hook", hook))
            hooks_mod.get_axon_ntff_profile_hook = (
                lambda: _holder.get("hook"))
            sys.modules["antenv.axon_hooks"] = hooks_mod
            antenv.axon_hooks = hooks_mod

        if hooks_mod.get_axon_ntff_profile_hook() is not None:
            return
        so_path = "/opt/axon/libaxon_pjrt.so"
        if not os.path.exists(so_path):
            return
        lib = ctypes.CDLL(so_path)
        if not hasattr(lib, "axon_start_nrt_profile"):
            return
        lib.axon_start_nrt_profile.argtypes = [
            ctypes.POINTER(ctypes.c_int64), ctypes.c_size_t]
        lib.axon_start_nrt_profile.restype = ctypes.c_int64
        lib.axon_stop_nrt_profile.argtypes = [ctypes.c_char_p]
        lib.axon_stop_nrt_profile.restype = ctypes.c_int64

        @contextlib.contextmanager
        def _hook(output_dir, device_ids):
            import jax

            jax.devices()
            if device_ids:
                ids = (ctypes.c_int64 * len(device_ids))(*device_ids)
                rc = lib.axon_start_nrt_profile(ids, len(device_ids))
            else:
                rc = lib.axon_start_nrt_profile(None, 0)
            if rc != 0:
                raise RuntimeError(f"axon_start_nrt_profile rc={rc}")
            try:
                yield
            finally:
                n = lib.axon_stop_nrt_profile(str(output_dir).encode())
                print(f"ntff profile: {n} file(s) written to {output_dir}")

        hooks_mod.set_axon_ntff_profile_hook(_hook)
    except Exception:
        pass


def _build_program():
    import concourse.mybir as mybir
    import concourse.tile as tile
    from concourse import bacc

    f16 = mybir.dt.float16
    f32 = mybir.dt.float32
    f8 = mybir.dt.float8e4
    Alu = mybir.AluOpType

    nc = bacc.Bacc(None, target_bir_lowering=False)

    x_st = nc.dram_tensor("x_st", [TPC, INT_F], f16, kind="ExternalInput")
    # fp8 stationary activations: [p, c, t] = fp_x[t, c*128+p]
    fpx_d = nc.dram_tensor("fpx", [128, 2, TPC], f8, kind="ExternalInput")
    # raw int4 weight values (exact in fp8e4m3), grouped k-major:
    # wq[g, p, e, o] = w[k = e*128+p, g*512+o]
    wq_d = nc.dram_tensor("wq", [NG, 128, KE - 1, OG], f8, kind="ExternalInput")
    # fp weights: fpw[p, c, o] = fpW[o, c*128+p]
    fpw_d = nc.dram_tensor("fpw", [128, 2, OUT_F], f8, kind="ExternalInput")
    wsb_d = nc.dram_tensor("wsb", [128, OUT_F], f16, kind="ExternalInput")
    # row0 = reduced_w, row1 = bias
    brw_d = nc.dram_tensor("brw", [2, OUT_F], f16, kind="ExternalInput")
    # row0 = zeros (alpha filled on device), row1 = ones
    ones2_d = nc.dram_tensor("ones2", [2, TPC], f16, kind="ExternalInput")
    out_d = nc.dram_tensor("out", [TPC, OUT_F], f16, kind="ExternalOutput")

    with tile.TileContext(nc) as tc:
        with tc.tile_pool(name="consts", bufs=1) as consts, \
             tc.tile_pool(name="xin", bufs=2) as xin, \
             tc.tile_pool(name="y0p", bufs=1) as y0p, \
             tc.tile_pool(name="qap", bufs=2) as qap, \
             tc.tile_pool(name="qtp", bufs=2) as qtp, \
             tc.tile_pool(name="qt8", bufs=NT) as qt8, \
             tc.tile_pool(name="wqp", bufs=5) as wqp, \
             tc.tile_pool(name="jnk", bufs=1) as jnk, \
             tc.tile_pool(name="stp", bufs=4) as stp, \
             tc.tile_pool(name="outp", bufs=2) as outp, \
             tc.tile_pool(name="ps0", bufs=4, space="PSUM") as ps0, \
             tc.tile_pool(name="ps1", bufs=4, space="PSUM") as ps1:

            # ALL loads ride one HWDGE ring (scalar queue), x tiles first:
            # the ring drains FIFO, which is the only reliable priority on
            # trn2, and keeping SWDGE quiet avoids DVE SBUF-port contention
            xts = []
            for r in range(NT):
                xt = xin.tile([128, INT_F], f16, tag=f"x{r}", bufs=1)
                nc.scalar.dma_start(
                    out=xt[:, :], in_=x_st[r * 128:(r + 1) * 128, :])
                xts.append(xt)

            # HWDGE (scalar) queue: two weight groups lead, then fp consts
            wq_tiles = []
            for g in range(2):
                wqg = wqp.tile([128, KE - 1, OG], f8)
                nc.scalar.dma_start(out=wqg[:, :, :], in_=wq_d[g, :, :, :])
                wq_tiles.append(wqg)

            fpx_s = consts.tile([128, 2, TPC], f8)
            nc.scalar.dma_start(out=fpx_s[:, :, :], in_=fpx_d[:, :, :])
            fpw_s = consts.tile([128, 2, OUT_F], f8)
            nc.scalar.dma_start(out=fpw_s[:, :, :], in_=fpw_d[:, :, :])
            wsb_s = consts.tile([128, OUT_F], f16)
            nc.scalar.dma_start(out=wsb_s[:, :], in_=wsb_d[:, :])
            brw_s = consts.tile([2, OUT_F], f16)
            nc.scalar.dma_start(out=brw_s[:, :], in_=brw_d[:, :])
            # row0 = alpha (filled per tile), row1 = ones (from DRAM:
            # engines cannot address partition base 1)
            onesal = consts.tile([2, TPC], f16)
            nc.scalar.dma_start(out=onesal[:, :], in_=ones2_d[:, :])

            for g in range(2, 4):
                wqg = wqp.tile([128, KE - 1, OG], f8)
                nc.scalar.dma_start(out=wqg[:, :, :], in_=wq_d[g, :, :, :])
                wq_tiles.append(wqg)

            # [scale, rs, bq, alpha] packed per tile r at ppack[:, 4r:4r+4]
            ppack = consts.tile([128, 4 * NT], f32)

            def param(r, v):
                idx = 4 * r + v
                return ppack[:, idx:idx + 1]

            # === Phase A, software-pipelined with an emission order that
            # avoids DVE head-of-line blocking: rounds for tile r are
            # interleaved between stats of tiles r+1/r+2, casts lag one
            # tile behind their transpose.
            def stats_params(r):
                xt = xts[r]
                mn = stp.tile([128, 1], f32, tag="mn")
                mx = stp.tile([128, 1], f32, tag="mx")
                a1 = jnk.tile([128, 1920], f16, tag="a1")
                a2 = jnk.tile([128, 960], f16, tag="a2")
                nc.vector.tensor_tensor(
                    out=a1[:, :], in0=xt[:, :1920], in1=xt[:, 1920:],
                    op=Alu.min)
                nc.vector.tensor_tensor(
                    out=a2[:, :], in0=a1[:, :960], in1=a1[:, 960:], op=Alu.min)
                nc.vector.tensor_reduce(
                    out=mn[:, :], in_=a2[:, :], axis=mybir.AxisListType.X,
                    op=Alu.min)
                b1 = jnk.tile([128, 1920], f16, tag="a1")
                b2 = jnk.tile([128, 960], f16, tag="a2")
                nc.vector.tensor_tensor(
                    out=b1[:, :], in0=xt[:, :1920], in1=xt[:, 1920:],
                    op=Alu.max)
                nc.vector.tensor_tensor(
                    out=b2[:, :], in0=b1[:, :960], in1=b1[:, 960:], op=Alu.max)
                nc.vector.tensor_reduce(
                    out=mx[:, :], in_=b2[:, :], axis=mybir.AxisListType.X,
                    op=Alu.max)
                d = stp.tile([128, 1], f32, tag="d")
                nc.vector.tensor_sub(d[:, :], mx[:, :], mn[:, :])
                nc.vector.tensor_scalar(
                    out=param(r, 0), in0=d[:, :],
                    scalar1=1.0 / 15.0, scalar2=1e-8, op0=Alu.mult, op1=Alu.max)
                nc.vector.reciprocal(param(r, 1), param(r, 0))
                tt = stp.tile([128, 1], f32, tag="tt")
                nc.vector.tensor_mul(tt[:, :], mn[:, :], param(r, 1))
                nc.vector.tensor_scalar(
                    out=param(r, 2), in0=tt[:, :],
                    scalar1=-1.0, scalar2=-8.0, op0=Alu.mult, op1=Alu.add)
                # alpha = mn + 8*scale (the zero-point term; rides the fp
                # psum which is NOT multiplied by scale_t in the combine)
                t8 = stp.tile([128, 1], f32, tag="t8")
                nc.vector.tensor_scalar(
                    out=t8[:, :], in0=param(r, 0),
                    scalar1=8.0, scalar2=None, op0=Alu.mult)
                nc.vector.tensor_add(param(r, 3), t8[:, :], mn[:, :])
                # quantize affine on ScalarE: y0 = x*rs + bq (f32: an f16
                # intermediate flips ~0.4% of q by +-1 and costs ~2.5x in
                # max error)
                y0 = y0p.tile([128, INT_F], f32)
                nc.scalar.activation(
                    out=y0[:, :], in_=xt[:, :],
                    func=mybir.ActivationFunctionType.Identity,
                    bias=param(r, 2), scale=param(r, 1))
                return y0

            qtbs = [None] * NT
            y0s = [None] * NT

            def round_transpose(r):
                y0 = y0s[r]
                qa = qap.tile([128, KP], f16)
                # q = (y0+C)-C : fp32-internal RNE round to integer
                nc.vector.tensor_scalar(
                    out=qa[:, :INT_F], in0=y0[:, :], scalar1=C_MAGIC,
                    scalar2=-C_MAGIC, op0=Alu.add, op1=Alu.add)
                # alpha column (transposes into a k-major row) + zero pad
                nc.vector.tensor_copy(
                    out=qa[:, INT_F:INT_F + 1], in_=param(r, 3))
                nc.vector.memset(qa[:, INT_F + 1:], 0.0)
                # k-major transpose via DMA xbar: qtb[p,e,t] = qa[t,e*128+p]
                qtb = qtp.tile([128, KE, 128], f16)
                nc.sync.dma_start_transpose(out=qtb[:, :, :], in_=qa[:, :])
                qtbs[r] = qtb

            q8s = [None] * NT

            def cast_alpha(r):
                qtb = qtbs[r]
                q8 = qt8.tile([128, KE - 1, 128], f8, tag=f"q8_{r}", bufs=1)
                nc.vector.tensor_copy(out=q8[:, :, :], in_=qtb[:, :KE - 1, :])
                # alpha row for the K=2 fp matmul
                nc.vector.tensor_copy(
                    out=onesal[0:1, r * 128:(r + 1) * 128],
                    in_=qtb[0:1, KE - 1, :])
                q8s[r] = q8

            y0s[0] = stats_params(0)
            y0s[1] = stats_params(1)
            round_transpose(0)
            y0s[2] = stats_params(2)
            round_transpose(1)
            cast_alpha(0)
            y0s[3] = stats_params(3)
            round_transpose(2)
            cast_alpha(1)
            round_transpose(3)
            cast_alpha(2)
            cast_alpha(3)

            # trailing weight groups: emitted after the ACT compute ops so
            # their ring-slot waits cannot block them
            for g in range(4, NG):
                wqg = wqp.tile([128, KE - 1, OG], f8)
                nc.scalar.dma_start(out=wqg[:, :, :], in_=wq_d[g, :, :, :])
                wq_tiles.append(wqg)

            # === Phase B: two cohorts of 4 out-feature groups; tiles
            # outer within a cohort so the first 4 blocks only need q8[0]
            def block(g, t):
                wqg = wq_tiles[g]
                o0 = g * OG
                t0 = t * 128
                p0 = ps0.tile([128, OG], f32)
                for e in range(15):
                    nc.tensor.matmul(
                        p0[:, :], q8s[t][:, 2 * e:2 * e + 2, :],
                        wqg[:, 2 * e:2 * e + 2, :],
                        start=(e == 0), stop=(e == 14),
                        perf_mode=mybir.MatmulPerfMode.DoubleRow)
                p1 = ps1.tile([128, OG], f32)
                nc.tensor.matmul(
                    p1[:, :], fpx_s[:, 0:2, t0:t0 + 128],
                    fpw_s[:, 0:2, o0:o0 + OG],
                    start=True, stop=False,
                    perf_mode=mybir.MatmulPerfMode.DoubleRow)
                # [alpha; ones].T @ [redw; bias] : K=2 f16 rank-2 update
                nc.tensor.matmul(
                    p1[:, :], onesal[0:2, t0:t0 + 128],
                    brw_s[0:2, o0:o0 + OG],
                    start=False, stop=True)
                # combine: out = (p0*wscale)*scale_t + p1
                m = outp.tile([128, OG], f32, tag="s1")
                nc.vector.tensor_mul(m[:, :], p0[:, :], wsb_s[:, o0:o0 + OG])
                ot = outp.tile([128, OG], f16, tag="ot")
                nc.vector.affine_then_add(
                    out=ot[:, :], in0=m[:, :], in1=p1[:, :],
                    scale=param(t, 0), bias=0.0)
                nc.gpsimd.dma_start(
                    out=out_d[t0:t0 + 128, o0:o0 + OG], in_=ot[:, :])

            for cohort in (range(0, 4), range(4, NG)):
                for t in range(NT):
                    for g in cohort:
                        block(g, t)

    nc.finalize()
    return nc


def _get_program():
    global _PROGRAM
    if _PROGRAM is None:
        _PROGRAM = _build_program()
    return _PROGRAM


def _unpack_i4(w_packed):
    """(out, INT_F//2) uint8 -> (out, INT_F) int8; col 2k=low nibble, 2k+1=high."""
    lo = (w_packed & 0x0F).astype(np.int8)
    hi = ((w_packed >> 4) & 0x0F).astype(np.int8)
    lo = np.where(lo >= 8, lo - 16, lo)
    hi = np.where(hi >= 8, hi - 16, hi)
    w = np.empty((w_packed.shape[0], w_packed.shape[1] * 2), dtype=np.int8)
    w[:, 0::2] = lo
    w[:, 1::2] = hi
    return w


def _prep_inputs(x, int_weight, weights_scales, reduced_w, fp_weight, bias,
                 int_indices, fp_indices):
    import ml_dtypes
    f8np = ml_dtypes.float8_e4m3

    x2 = np.asarray(x, dtype=np.float16)[0]
    int_idx = np.asarray(int_indices).astype(np.int64)
    fp_idx = np.asarray(fp_indices).astype(np.int64)

    x_int = np.ascontiguousarray(x2[:, int_idx])            # (M, 3840) f16
    fp_xT = np.ascontiguousarray(x2[:, fp_idx].T)           # (256, M) f16

    w_int = _unpack_i4(np.asarray(int_weight))              # (OUT_F, 3840) int8
    # wq[g, p, e, o] = w[k=e*128+p, o=g*512+o']  (raw int4 vals, fp8 exact)
    wq = np.ascontiguousarray(
        w_int.T.reshape(KE - 1, 128, NG, OG).transpose(2, 1, 0, 3)
    ).astype(f8np)

    wsc = np.asarray(weights_scales).astype(np.float16)     # (OUT_F, 1)
    wsb = np.broadcast_to(wsc[:, 0][None, :], (128, OUT_F)).copy()

    fpW = np.asarray(fp_weight).astype(np.float16)          # (OUT_F, 256)
    fpw = np.ascontiguousarray(
        fpW.T.reshape(2, 128, OUT_F).transpose(1, 0, 2)).astype(f8np)

    redw = np.asarray(reduced_w).astype(np.float16)         # (1, OUT_F)
    b = np.asarray(bias).astype(np.float16)                 # (OUT_F,)
    brw = np.stack([redw[0], b])                            # (2, OUT_F) f16
    ones2 = np.zeros((2, TPC), dtype=np.float16)
    ones2[1, :] = 1.0

    in_maps = []
    for c in range(NCORES):
        tok = slice(c * TPC, (c + 1) * TPC)
        x_stc = x_int[tok]                                  # contiguous view
        fpx = np.ascontiguousarray(
            fp_xT[:, tok].reshape(2, 128, TPC).transpose(1, 0, 2)
        ).astype(f8np)
        in_maps.append({"x_st": x_stc, "fpx": fpx, "wq": wq,
                        "fpw": fpw, "wsb": wsb, "brw": brw, "ones2": ones2})
    return in_maps


def kernel(x, int_weight, weights_scales, reduced_w, fp_weight, bias,
           int_indices, fp_indices):
    global LAST_RESULTS
    from concourse.bass_utils import run_bass_kernel_spmd

    _ensure_ntff_hook()
    in_maps = _prep_inputs(x, int_weight, weights_scales, reduced_w,
                           fp_weight, bias, int_indices, fp_indices)
    nc = _get_program()
    res = run_bass_kernel_spmd(nc, in_maps, core_ids=list(range(NCORES)))
    LAST_RESULTS = res
    out = np.concatenate([res.results[c]["out"] for c in range(NCORES)], axis=0)
    return out[None].astype(np.float16)

